# revision 1
# baseline (speedup 1.0000x reference)
"""Expert-parallel MoE layer for Trainium2 (8 NeuronCores, one expert per core).

Host side (numpy): router logits, exact top-2 dispatch, p0 weights, and the
scatter-add combine. Device side (Bass/Tile, SPMD over 8 cores): the dense FFN
y = gelu(x @ W1[e] + b1[e]) @ W2[e] over the tokens routed to expert e,
computed with fp16 operands (fp32 PSUM accumulation).

Per-core layout: F (the 4096-wide hidden dim) is processed in NQ=4 quarters
with W1/W2 quarter-slices resident in SBUF (double-buffered so the next
quarter's weights prefetch during compute); the whole fp16 xT stays resident;
y partials accumulate across quarters via DMA-accumulate into DRAM.
"""

import numpy as np

B, S, H, E, F = 4, 2048, 1024, 8, 4096
T = B * S
P = 128
NQ = 4              # F quarters (outer loop); W1q + W2q resident per quarter
FQ = F // NQ
TT = 512            # token group (GEMM1 moving free dim)
MIN_CAP = 2304      # >= max per-expert load for the fixed seed-0 input (~2182)

_cache = {}


def _spill_waits(nc, mybir, max_waits=1):
    """walrus CoreV2/V3 codegen rejects instructions with >1 semaphore wait
    ("Too many sync wait commands") — notably self-loading fp32/fp32r matmuls
    and DMACopy. Move excess waits onto same-engine no-ops inserted right
    before the instruction (sequencers run in order, so this is equivalent)."""
    for fn in nc.m.functions:
        for blk in fn.blocks:
            out = []
            changed = False
            for inst in blk.instructions:
                si = getattr(inst, "sync_info", None)
                if si is not None and len(si.on_wait) > max_waits:
                    spill = si.on_wait[: len(si.on_wait) - max_waits]
                    keep = si.on_wait[len(si.on_wait) - max_waits:]
                    for w in spill:
                        nop = mybir.InstNoOp(
                            name=nc.get_next_instruction_name(),
                            engine=inst.engine,
                            ins=[],
                            outs=[],
                        )
                        nop.sync_info = mybir.SyncInfo(on_wait=[w], on_update=[])
                        out.append(nop)
                    inst.sync_info = mybir.SyncInfo(on_wait=keep, on_update=si.on_update)
                    changed = True
                out.append(inst)
            if changed:
                blk.instructions = out


def _build(cap, w1_bufs=2):
    import concourse.bass as bass
    import concourse.mybir as mybir
    from concourse import tile

    F32 = mybir.dt.float32
    MDT = mybir.dt.float32r
    # all matmul operands fp16 (walrus forbids mixing fp32r with others):
    # halves DMA bytes + LDWEIGHTS time at ~2^-11 quantization cost
    SDT = mybir.dt.float16
    GELU = mybir.ActivationFunctionType.Gelu_apprx_tanh
    ADD = mybir.AluOpType.add

    nc = bass.Bass()
    xt = nc.declare_dram_parameter("xt", [H, cap], SDT, isOutput=False)
    w1 = nc.declare_dram_parameter("w1", [H, F], SDT, isOutput=False)
    w2 = nc.declare_dram_parameter("w2", [F, H], SDT, isOutput=False)
    b1s = nc.declare_dram_parameter("b1s", [P, F // P], F32, isOutput=False)
    y = nc.declare_dram_parameter("y", [cap, H], F32, isOutput=True)

    KH = H // P          # k-chunks over H (GEMM1 contraction)
    KFQ = FQ // P        # k-chunks over one F quarter (GEMM2 contraction)
    n_rows = cap // P
    groups = []
    o = 0
    while o < cap:
        tt = min(TT, cap - o)
        groups.append((o, tt))
        o += tt

    with tile.TileContext(nc) as tc:
        with (
            tc.tile_pool(name="w1p", bufs=w1_bufs) as w1p,
            tc.tile_pool(name="w2p", bufs=2) as w2p,
            tc.tile_pool(name="xp", bufs=1) as xp,
            tc.tile_pool(name="hp", bufs=1) as hp,
            tc.tile_pool(name="yp", bufs=1) as yp,
            tc.tile_pool(name="cst", bufs=1) as cst,
            tc.tile_pool(name="ps1", bufs=4, space="PSUM") as ps1,
            tc.tile_pool(name="ps2", bufs=4, space="PSUM") as ps2,
        ):
            def load_w1(q, split=False):
                # split=True (startup only): f-major halves on both HWDGE
                # fifos — the first half covers every k-chunk for fs=0..3, so
                # GEMM1's first four accumulation groups (32 matmuls, ~7us)
                # run while the second half is still in flight
                w1q = w1p.tile([P, KH, FQ], SDT, tag="w1q")
                src = w1[:, q * FQ:(q + 1) * FQ].rearrange("(c p) f -> p c f", p=P)
                if split:
                    nc.sync.dma_start(w1q[:, : KH // 2, :], src[:, : KH // 2, :])
                    nc.scalar.dma_start(w1q[:, KH // 2:, :], src[:, KH // 2:, :])
                else:
                    nc.sync.dma_start(w1q[:], src)
                return w1q

            def load_w2(q, split=False):
                # startup split is n-major: first half serves all n=0 output
                # tiles of GEMM2 for every k2
                w2q = w2p.tile([P, KFQ, H], SDT, tag="w2q")
                src = w2[q * FQ:(q + 1) * FQ, :].rearrange("(c p) h -> p c h", p=P)
                if split:
                    nc.scalar.dma_start(w2q[:, : KFQ // 2, :], src[:, : KFQ // 2, :])
                    nc.sync.dma_start(w2q[:, KFQ // 2:, :], src[:, KFQ // 2:, :])
                else:
                    nc.scalar.dma_start(w2q[:], src)
                return w2q

            # prologue: bias (tiny) + the first token group of x ahead of
            # the W1 halves; the rest of x (resident for the whole kernel in
            # fp16) follows once the startup-critical loads are queued
            b1t = cst.tile([P, F // P], F32)
            nc.scalar.dma_start(b1t[:], b1s[:])
            x_all = xp.tile([P, KH, cap], SDT)
            xsrc = xt.rearrange("(c p) t -> p c t", p=P)
            nc.scalar.dma_start(x_all[:, :, :TT], xsrc[:, :, :TT])
            w1q = load_w1(0, split=True)
            nc.scalar.dma_start(x_all[:, :, TT:], xsrc[:, :, TT:])
            w2q = None
            for q in range(NQ):
                for gi, (t0, tt) in enumerate(groups):
                    # GEMM1: hT[f, t] = sum_h W1[h, f] * xT[h, t], then gelu
                    hq = hp.tile([P, KFQ, TT], SDT, tag="hq")
                    for fs in range(KFQ):
                        pt = ps1.tile([P, TT], F32, tag="pt1")
                        for k in range(KH):
                            nc.tensor.matmul(
                                pt[:, :tt],
                                w1q[:, k, fs * P:(fs + 1) * P],
                                x_all[:, k, t0:t0 + tt],
                                start=(k == 0),
                                stop=(k == KH - 1),
                            )
                        c = q * KFQ + fs
                        nc.scalar.activation(
                            hq[:, fs, :tt], pt[:, :tt], GELU, bias=b1t[:, c:c + 1]
                        )
                    if q == 0 and gi == 0:
                        # W2 deliberately after GEMM1(group 0): its first use
                        # is GEMM2, so don't let it contend with W1/x at start
                        w2q = load_w2(0, split=True)
                    if gi == 0 and q + 1 < NQ:
                        w1_nxt = load_w1(q + 1)
                    if gi == 2 and q + 1 < NQ:
                        w2_nxt = load_w2(q + 1)
                    # GEMM2: y[t, h'] += sum_f hT[f, t] * W2[f, h']
                    rows = tt // P
                    stage = yp.tile([P, TT // P, H], F32, tag="stage")
                    for ms in range(rows):
                        for n in range(H // 512):
                            pt2 = ps2.tile([P, 512], F32, tag="pt2")
                            for k2 in range(KFQ):
                                nc.tensor.matmul(
                                    pt2[:],
                                    hq[:, k2, ms * P:(ms + 1) * P],
                                    w2q[:, k2, n * 512:(n + 1) * 512],
                                    start=(k2 == 0),
                                    stop=(k2 == KFQ - 1),
                                )
                            nc.vector.tensor_copy(
                                stage[:, ms, n * 512:(n + 1) * 512], pt2[:]
                            )
                    # y partial for this (quarter, group): write (q==0) or
                    # DMA-accumulate (q>0) into the y DRAM buffer
                    r0 = t0 // P
                    ydram = y.rearrange("(j p) h -> p j h", p=P)
                    if q == 0:
                        nc.sync.dma_start(
                            ydram[:, r0:r0 + rows, :], stage[:, :rows, :]
                        )
                    elif q == NQ - 1 and gi == len(groups) - 1:
                        # final writeback: per-row accum DMAs so most of it
                        # drains while the last matmul group still runs
                        for ms in range(rows):
                            nc.gpsimd.dma_start(
                                ydram[:, r0 + ms:r0 + ms + 1, :],
                                stage[:, ms:ms + 1, :],
                                accum_op=ADD,
                            )
                    else:
                        nc.gpsimd.dma_start(
                            ydram[:, r0:r0 + rows, :], stage[:, :rows, :], accum_op=ADD
                        )
                if q + 1 < NQ:
                    w1q, w2q = w1_nxt, w2_nxt

    import concourse.mybir as mybir_mod

    _spill_waits(nc, mybir_mod)
    return nc


def _route(x2d, Wr, br):
    """Top-2 routing, bit-matching the reference's decisions.

    Softmax is monotonic, so top-2-of-probs == top-2-of-logits, and the
    normalized top-1 weight p0 = p1/(p1+p2) == sigmoid(l1-l2) exactly (the
    softmax denominator cancels). Ordering ties are broken by lower index,
    same as jax.lax.top_k."""
    logits = x2d @ np.asarray(Wr, np.float32) + np.asarray(br, np.float32)
    order = np.argsort(-logits, axis=-1, kind="stable")
    i1 = order[:, 0].astype(np.int64)
    i2 = order[:, 1].astype(np.int64)
    r = np.arange(logits.shape[0])
    l1 = logits[r, i1].astype(np.float64)
    l2 = logits[r, i2].astype(np.float64)
    p0 = 1.0 / (1.0 + np.exp(l2 - l1))
    return i1, i2, p0.astype(np.float32)


def kernel(x, Wr, br, W1, b1, W2, b2):
    from concourse.bass_utils import run_bass_kernel_spmd

    x2d = np.ascontiguousarray(np.asarray(x, np.float32).reshape(T, H))
    W1 = np.asarray(W1, np.float32)
    b1 = np.asarray(b1, np.float32)
    W2 = np.asarray(W2, np.float32)
    b2 = np.asarray(b2, np.float32)

    i1, i2, p0 = _route(x2d, Wr, br)

    idxs = [np.flatnonzero((i1 == e) | (i2 == e)) for e in range(E)]
    max_cnt = max(len(ix) for ix in idxs)
    cap = max(MIN_CAP, -(-max_cnt // 256) * 256)

    key = cap
    if key not in _cache:
        _cache[key] = _build(cap)
    nc = _cache[key]

    xT = np.ascontiguousarray(x2d.T)  # [H, T]
    in_maps = []
    for e in range(E):
        ix = idxs[e]
        xte = np.zeros((H, cap), np.float32)
        xte[:, : len(ix)] = xT[:, ix]
        b1se = np.ascontiguousarray(b1[e].reshape(F // P, P).T)
        in_maps.append(
            {
                "xt": xte.astype(np.float16),
                "w1": np.ascontiguousarray(W1[e]).astype(np.float16),
                "w2": np.ascontiguousarray(W2[e]).astype(np.float16),
                "b1s": b1se,
            }
        )

    try:
        res = run_bass_kernel_spmd(nc, in_maps, list(range(E)))
    except Exception:
        import time as _time

        _time.sleep(10)
        res = run_bass_kernel_spmd(nc, in_maps, list(range(E)))

    out = np.zeros((T, H), np.float32)
    for e in range(E):
        ix = idxs[e]
        ye = res.results[e]["y"][: len(ix)]
        out[ix] += p0[ix, None] * (ye + b2[e][None, :])
    return out.reshape(B, S, H)



# revision 2
# speedup vs baseline: 1.0016x; 1.0016x over previous
"""Expert-parallel MoE layer for Trainium2 (8 NeuronCores, one expert per core).

Host side (numpy): router logits, exact top-2 dispatch, p0 weights, and the
scatter-add combine. Device side (Bass/Tile, SPMD over 8 cores): the dense FFN
y = gelu(x @ W1[e] + b1[e]) @ W2[e] over the tokens routed to expert e,
computed with fp16 operands (fp32 PSUM accumulation).

v2 layout: tokens ride the MOVING dim of BOTH GEMMs, so the per-core token
capacity is the exact max expert load (no 128-row padding), and GEMM2 consumes
h^T [F, tokens] directly, producing y^T [H, tokens] written to DRAM exactly
once (no DRAM read-modify-write accumulation like v1's quartered scheme).
Full W1 and W2 stay SBUF-resident in fp16 (128 KB/partition of the 224 KB).

Per token group (TT=512, tail-sized last group):
  GEMM1: psum[f128, t] = sum_k w1[k, f128]^T x^T[k, t]   (8 k-chunks over H)
  gelu+bias -> h[f128-chunk, t] fp16                     (32 f-chunks)
  GEMM2: psum[h'128, t] = sum_k2 w2[k2, h'128]^T h[k2, t] (32 k2-chunks over F)
  copy -> y^T stage -> single DMA store per group
"""

import numpy as np

B, S, H, E, F = 4, 2048, 1024, 8, 4096
T = B * S
P = 128
TT = 512            # token group size (moving free dim of both GEMMs)
KH = H // P         # 8  k-chunks over H  (GEMM1 contraction)
KF = F // P         # 32 k-chunks over F  (GEMM2 contraction)
NH = H // P         # 8  output h'-chunks of GEMM2

_cache = {}


def _spill_waits(nc, mybir, max_waits=1):
    """walrus CoreV2/V3 codegen rejects instructions with >1 semaphore wait
    ("Too many sync wait commands"). Move excess waits onto same-engine no-ops
    inserted right before the instruction (sequencers run in order, so this is
    equivalent)."""
    for fn in nc.m.functions:
        for blk in fn.blocks:
            out = []
            changed = False
            for inst in blk.instructions:
                si = getattr(inst, "sync_info", None)
                if si is not None and len(si.on_wait) > max_waits:
                    spill = si.on_wait[: len(si.on_wait) - max_waits]
                    keep = si.on_wait[len(si.on_wait) - max_waits:]
                    for w in spill:
                        nop = mybir.InstNoOp(
                            name=nc.get_next_instruction_name(),
                            engine=inst.engine,
                            ins=[],
                            outs=[],
                        )
                        nop.sync_info = mybir.SyncInfo(on_wait=[w], on_update=[])
                        out.append(nop)
                    inst.sync_info = mybir.SyncInfo(on_wait=keep, on_update=si.on_update)
                    changed = True
                out.append(inst)
            if changed:
                blk.instructions = out


def _build(cap):
    import concourse.bass as bass
    import concourse.mybir as mybir
    from concourse import tile

    F32 = mybir.dt.float32
    SDT = mybir.dt.float16
    GELU = mybir.ActivationFunctionType.Gelu_apprx_tanh

    nc = bass.Bass()
    xt = nc.declare_dram_parameter("xt", [H, cap], SDT, isOutput=False)
    w1 = nc.declare_dram_parameter("w1", [H, F], SDT, isOutput=False)
    w2 = nc.declare_dram_parameter("w2", [F, H], SDT, isOutput=False)
    b1s = nc.declare_dram_parameter("b1s", [P, KF], F32, isOutput=False)
    yt = nc.declare_dram_parameter("yt", [H, cap], F32, isOutput=True)

    # token groups: full TT groups first, tail last (w1/w2 stream during g0)
    groups = []
    o = 0
    while o < cap:
        tt = min(TT, cap - o)
        groups.append((o, tt))
        o += tt

    xsrc = xt.rearrange("(c p) t -> p c t", p=P)
    w1src = w1.rearrange("(c p) f -> p c f", p=P)
    w2src = w2.rearrange("(c p) h -> p c h", p=P)
    ydst = yt.rearrange("(c p) t -> p c t", p=P)

    with tile.TileContext(nc) as tc:
        with (
            tc.tile_pool(name="w1p", bufs=1) as w1p,
            tc.tile_pool(name="w2p", bufs=1) as w2p,
            tc.tile_pool(name="xp", bufs=2) as xp,
            tc.tile_pool(name="hp", bufs=1) as hp,
            tc.tile_pool(name="yp", bufs=1) as yp,
            tc.tile_pool(name="cst", bufs=1) as cst,
            tc.tile_pool(name="ps1", bufs=4, space="PSUM") as ps1,
            tc.tile_pool(name="ps2", bufs=4, space="PSUM") as ps2,
        ):
            b1t = cst.tile([P, KF], F32)
            nc.scalar.dma_start(b1t[:], b1s[:])

            # x group 0 first (first GEMM needs it), then W1 in fs-quarter-major
            # chunks so the fs=0..7 psum groups unblock after ~1/4 of the W1
            # stream, then the rest. W2 on the other HWDGE ring; it is first
            # used ~55us in and completes by ~25us.
            def load_x(gi):
                t0, tt = groups[gi]
                xg = xp.tile([P, KH, TT], SDT, tag="xg")
                nc.scalar.dma_start(xg[:, :, :tt], xsrc[:, :, t0:t0 + tt])
                return xg

            xg = load_x(0)

            w1r = w1p.tile([P, KH, F], SDT, tag="w1r")
            for fq in range(4):
                fl = fq * (F // 4)
                fh = fl + F // 4
                for k in range(KH):
                    nc.scalar.dma_start(
                        w1r[:, k, fl:fh], w1src[:, k, fl:fh]
                    )
            w2r = w2p.tile([P, KF, H], SDT, tag="w2r")
            nc.sync.dma_start(w2r[:], w2src[:])

            h = hp.tile([P, KF, TT], SDT, tag="h")
            for gi, (t0, tt) in enumerate(groups):
                if gi + 1 < len(groups):
                    xg_nxt = load_x(gi + 1)
                # GEMM1: h^T[f, t] = gelu(sum_k W1[k, f] * x^T[k, t] + b1[f])
                for fs in range(KF):
                    pt = ps1.tile([P, TT], F32, tag="pt1")
                    for k in range(KH):
                        nc.tensor.matmul(
                            pt[:, :tt],
                            w1r[:, k, fs * P:(fs + 1) * P],
                            xg[:, k, :tt],
                            start=(k == 0),
                            stop=(k == KH - 1),
                        )
                    nc.scalar.activation(
                        h[:, fs, :tt], pt[:, :tt], GELU, bias=b1t[:, fs:fs + 1]
                    )
                # GEMM2: y^T[h', t] = sum_k2 W2[k2, h'] * h^T[k2, t]
                stage = yp.tile([P, NH, TT], F32, tag="stage")
                for n in range(NH):
                    pt2 = ps2.tile([P, TT], F32, tag="pt2")
                    for k2 in range(KF):
                        nc.tensor.matmul(
                            pt2[:, :tt],
                            w2r[:, k2, n * P:(n + 1) * P],
                            h[:, k2, :tt],
                            start=(k2 == 0),
                            stop=(k2 == KF - 1),
                        )
                    nc.vector.tensor_copy(stage[:, n, :tt], pt2[:, :tt])
                nc.sync.dma_start(ydst[:, :, t0:t0 + tt], stage[:, :, :tt])
                xg = xg_nxt if gi + 1 < len(groups) else None

    import concourse.mybir as mybir_mod

    _spill_waits(nc, mybir_mod)
    return nc


def _route(x2d, Wr, br):
    """Top-2 routing, bit-matching the reference's decisions.

    Softmax is monotonic, so top-2-of-probs == top-2-of-logits, and the
    normalized top-1 weight p0 = p1/(p1+p2) == sigmoid(l1-l2) exactly (the
    softmax denominator cancels). Ordering ties are broken by lower index,
    same as jax.lax.top_k."""
    logits = x2d @ np.asarray(Wr, np.float32) + np.asarray(br, np.float32)
    order = np.argsort(-logits, axis=-1, kind="stable")
    i1 = order[:, 0].astype(np.int64)
    i2 = order[:, 1].astype(np.int64)
    r = np.arange(logits.shape[0])
    l1 = logits[r, i1].astype(np.float64)
    l2 = logits[r, i2].astype(np.float64)
    p0 = 1.0 / (1.0 + np.exp(l2 - l1))
    return i1, i2, p0.astype(np.float32)


def _prepare(x, Wr, br, W1, b1, W2, b2):
    """Route on host, build per-core input maps and the (cached) Bass program."""
    x2d = np.ascontiguousarray(np.asarray(x, np.float32).reshape(T, H))
    W1 = np.asarray(W1, np.float32)
    b1 = np.asarray(b1, np.float32)
    W2 = np.asarray(W2, np.float32)

    i1, i2, p0 = _route(x2d, Wr, br)
    idxs = [np.flatnonzero((i1 == e) | (i2 == e)) for e in range(E)]
    cap = max(len(ix) for ix in idxs)

    if cap not in _cache:
        _cache[cap] = _build(cap)
    nc = _cache[cap]

    xT = np.ascontiguousarray(x2d.T)  # [H, T]
    in_maps = []
    for e in range(E):
        ix = idxs[e]
        xte = np.zeros((H, cap), np.float16)
        xte[:, : len(ix)] = xT[:, ix]
        b1se = np.ascontiguousarray(b1[e].reshape(KF, P).T)
        in_maps.append(
            {
                "xt": xte,
                "w1": np.ascontiguousarray(W1[e]).astype(np.float16),
                "w2": np.ascontiguousarray(W2[e]).astype(np.float16),
                "b1s": b1se,
            }
        )
    return nc, in_maps, idxs, p0


def _combine(res, idxs, p0, b2):
    b2 = np.asarray(b2, np.float32)
    out = np.zeros((T, H), np.float32)
    for e in range(E):
        ix = idxs[e]
        ye = res.results[e]["yt"][:, : len(ix)].T  # [n_e, H]
        out[ix] += p0[ix, None] * (ye + b2[e][None, :])
    return out.reshape(B, S, H)


def kernel(x, Wr, br, W1, b1, W2, b2):
    from concourse.bass_utils import run_bass_kernel_spmd

    nc, in_maps, idxs, p0 = _prepare(x, Wr, br, W1, b1, W2, b2)
    try:
        res = run_bass_kernel_spmd(nc, in_maps, list(range(E)))
    except Exception:
        import time as _time

        _time.sleep(10)
        res = run_bass_kernel_spmd(nc, in_maps, list(range(E)))
    return _combine(res, idxs, p0, b2)


# revision 5
# speedup vs baseline: 1.0505x; 1.0488x over previous
"""Expert-parallel MoE layer for Trainium2 (8 NeuronCores, one expert per core).

Host side (numpy): router logits, exact top-2 dispatch, p0 weights, and the
scatter-add combine. Device side (Bass/Tile, SPMD over 8 cores): the dense FFN
y = gelu(x @ W1[e] + b1[e]) @ W2[e] over the tokens routed to expert e,
computed with fp16 operands (fp32 PSUM accumulation).

v2 layout: tokens ride the MOVING dim of BOTH GEMMs, so the per-core token
capacity is the exact max expert load (no 128-row padding), and GEMM2 consumes
h^T [F, tokens] directly, producing y^T [H, tokens] written to DRAM exactly
once (no DRAM read-modify-write accumulation like v1's quartered scheme).
Full W1 and W2 stay SBUF-resident in fp16 (128 KB/partition of the 224 KB).

Per token group (TT=512, tail-sized last group):
  GEMM1: psum[f128, t] = sum_k w1[k, f128]^T x^T[k, t]   (8 k-chunks over H)
  gelu+bias -> h[f128-chunk, t] fp16                     (32 f-chunks)
  GEMM2: psum[h'128, t] = sum_k2 w2[k2, h'128]^T h[k2, t] (32 k2-chunks over F)
  copy -> y^T stage -> single DMA store per group
"""

import numpy as np

B, S, H, E, F = 4, 2048, 1024, 8, 4096
T = B * S
P = 128
TT = 512            # token group size (moving free dim of both GEMMs)
KH = H // P         # 8  k-chunks over H  (GEMM1 contraction)
KF = F // P         # 32 k-chunks over F  (GEMM2 contraction)
NH = H // P         # 8  output h'-chunks of GEMM2

_cache = {}


def _spill_waits(nc, mybir, max_waits=1):
    """walrus CoreV2/V3 codegen rejects instructions with >1 semaphore wait
    ("Too many sync wait commands"). Move excess waits onto same-engine no-ops
    inserted right before the instruction (sequencers run in order, so this is
    equivalent)."""
    for fn in nc.m.functions:
        for blk in fn.blocks:
            out = []
            changed = False
            for inst in blk.instructions:
                si = getattr(inst, "sync_info", None)
                if si is not None and len(si.on_wait) > max_waits:
                    spill = si.on_wait[: len(si.on_wait) - max_waits]
                    keep = si.on_wait[len(si.on_wait) - max_waits:]
                    for w in spill:
                        nop = mybir.InstNoOp(
                            name=nc.get_next_instruction_name(),
                            engine=inst.engine,
                            ins=[],
                            outs=[],
                        )
                        nop.sync_info = mybir.SyncInfo(on_wait=[w], on_update=[])
                        out.append(nop)
                    inst.sync_info = mybir.SyncInfo(on_wait=keep, on_update=si.on_update)
                    changed = True
                out.append(inst)
            if changed:
                blk.instructions = out


def _build(cap):
    import concourse.bass as bass
    import concourse.mybir as mybir
    from concourse import tile

    F32 = mybir.dt.float32
    SDT = mybir.dt.float16
    GELU = mybir.ActivationFunctionType.Gelu_apprx_tanh

    nc = bass.Bass()
    xt = nc.declare_dram_parameter("xt", [H, cap], SDT, isOutput=False)
    w1 = nc.declare_dram_parameter("w1", [H, F], SDT, isOutput=False)
    w2 = nc.declare_dram_parameter("w2", [F, H], SDT, isOutput=False)
    b1s = nc.declare_dram_parameter("b1s", [P, KF], F32, isOutput=False)
    yt = nc.declare_dram_parameter("yt", [H, cap], F32, isOutput=True)

    # token groups: full TT groups first, tail last (w1/w2 stream during g0)
    groups = []
    o = 0
    while o < cap:
        tt = min(TT, cap - o)
        groups.append((o, tt))
        o += tt

    xsrc = xt.rearrange("(c p) t -> p c t", p=P)
    w1src = w1.rearrange("(c p) f -> p c f", p=P)
    w2src = w2.rearrange("(c p) h -> p c h", p=P)
    ydst = yt.rearrange("(c p) t -> p c t", p=P)

    with tile.TileContext(nc) as tc:
        with (
            tc.tile_pool(name="w1p", bufs=1) as w1p,
            tc.tile_pool(name="w2p", bufs=1) as w2p,
            tc.tile_pool(name="xp", bufs=1) as xp,
            tc.tile_pool(name="hp", bufs=1) as hp,
            tc.tile_pool(name="yp", bufs=4) as yp,
            tc.tile_pool(name="cst", bufs=1) as cst,
            tc.tile_pool(name="ps1", bufs=4, space="PSUM") as ps1,
            tc.tile_pool(name="ps2", bufs=4, space="PSUM") as ps2,
        ):
            # Startup DMA orchestration. Tile hands DMA-completion semaphore
            # lanes out of a shared pool of 8, so a single huge DMA parks a
            # lane for its whole transfer and stalls later DMAs that need the
            # lane back -- keep every DMA around ~1 MB. Consumption order is
            # w1 quarter 0..3 (fs ascending), then w2 (first GEMM2 ~55us in).
            # Split across both HWDGE rings: sync gets quarters 0,2 + w2,
            # scalar gets x + quarters 1,3 (x group 0 lands ~10us, in time
            # for the first matmul).
            w1r = w1p.tile([P, KH, F], SDT, tag="w1r")
            x_all = xp.tile([P, KH, cap], SDT, tag="x")
            b1t = cst.tile([P, KF], F32)
            w2r = w2p.tile([P, KF, H], SDT, tag="w2r")

            FQ = F // 4
            for k in range(KH):
                nc.sync.dma_start(w1r[:, k, 0:FQ], w1src[:, k, 0:FQ])
            nc.scalar.dma_start(x_all[:, :, :TT], xsrc[:, :, :TT])
            for k in range(KH):
                nc.scalar.dma_start(
                    w1r[:, k, FQ:2 * FQ], w1src[:, k, FQ:2 * FQ]
                )
            for k in range(KH):
                nc.sync.dma_start(
                    w1r[:, k, 2 * FQ:3 * FQ], w1src[:, k, 2 * FQ:3 * FQ]
                )
            for k in range(KH):
                nc.scalar.dma_start(
                    w1r[:, k, 3 * FQ:4 * FQ], w1src[:, k, 3 * FQ:4 * FQ]
                )
            nc.sync.dma_start(b1t[:], b1s[:])
            for kc in range(0, KF, 4):
                nc.sync.dma_start(
                    w2r[:, kc:kc + 4, :], w2src[:, kc:kc + 4, :]
                )
            rest = (cap - TT + 1) // 2
            nc.scalar.dma_start(
                x_all[:, :, TT:TT + rest], xsrc[:, :, TT:TT + rest]
            )
            nc.scalar.dma_start(
                x_all[:, :, TT + rest:], xsrc[:, :, TT + rest:]
            )

            h = hp.tile([P, KF, TT], SDT, tag="h")
            for gi, (t0, tt) in enumerate(groups):
                # GEMM1: h^T[f, t] = gelu(sum_k W1[k, f] * x^T[k, t] + b1[f])
                for fs in range(KF):
                    pt = ps1.tile([P, TT], F32, tag="pt1")
                    for k in range(KH):
                        nc.tensor.matmul(
                            pt[:, :tt],
                            w1r[:, k, fs * P:(fs + 1) * P],
                            x_all[:, k, t0:t0 + tt],
                            start=(k == 0),
                            stop=(k == KH - 1),
                        )
                    nc.scalar.activation(
                        h[:, fs, :tt], pt[:, :tt], GELU, bias=b1t[:, fs:fs + 1]
                    )
                # GEMM2: y^T[h', t] = sum_k2 W2[k2, h'] * h^T[k2, t]
                # per-h'-chunk staging+store so the tail drains during the
                # last copies (and the stage stays at 4x2KB of SBUF)
                for n in range(NH):
                    pt2 = ps2.tile([P, TT], F32, tag="pt2")
                    for k2 in range(KF):
                        nc.tensor.matmul(
                            pt2[:, :tt],
                            w2r[:, k2, n * P:(n + 1) * P],
                            h[:, k2, :tt],
                            start=(k2 == 0),
                            stop=(k2 == KF - 1),
                        )
                    stage = yp.tile([P, TT], F32, tag="stage")
                    nc.vector.tensor_copy(stage[:, :tt], pt2[:, :tt])
                    nc.sync.dma_start(
                        ydst[:, n, t0:t0 + tt], stage[:, :tt]
                    )

    import concourse.mybir as mybir_mod

    _spill_waits(nc, mybir_mod)
    return nc


def _route(x2d, Wr, br):
    """Top-2 routing, bit-matching the reference's decisions.

    Softmax is monotonic, so top-2-of-probs == top-2-of-logits, and the
    normalized top-1 weight p0 = p1/(p1+p2) == sigmoid(l1-l2) exactly (the
    softmax denominator cancels). Ordering ties are broken by lower index,
    same as jax.lax.top_k."""
    logits = x2d @ np.asarray(Wr, np.float32) + np.asarray(br, np.float32)
    order = np.argsort(-logits, axis=-1, kind="stable")
    i1 = order[:, 0].astype(np.int64)
    i2 = order[:, 1].astype(np.int64)
    r = np.arange(logits.shape[0])
    l1 = logits[r, i1].astype(np.float64)
    l2 = logits[r, i2].astype(np.float64)
    p0 = 1.0 / (1.0 + np.exp(l2 - l1))
    return i1, i2, p0.astype(np.float32)


def _prepare(x, Wr, br, W1, b1, W2, b2):
    """Route on host, build per-core input maps and the (cached) Bass program."""
    x2d = np.ascontiguousarray(np.asarray(x, np.float32).reshape(T, H))
    W1 = np.asarray(W1, np.float32)
    b1 = np.asarray(b1, np.float32)
    W2 = np.asarray(W2, np.float32)

    i1, i2, p0 = _route(x2d, Wr, br)
    idxs = [np.flatnonzero((i1 == e) | (i2 == e)) for e in range(E)]
    cap = max(len(ix) for ix in idxs)

    if cap not in _cache:
        _cache[cap] = _build(cap)
    nc = _cache[cap]

    xT = np.ascontiguousarray(x2d.T)  # [H, T]
    in_maps = []
    for e in range(E):
        ix = idxs[e]
        xte = np.zeros((H, cap), np.float16)
        xte[:, : len(ix)] = xT[:, ix]
        b1se = np.ascontiguousarray(b1[e].reshape(KF, P).T)
        in_maps.append(
            {
                "xt": xte,
                "w1": np.ascontiguousarray(W1[e]).astype(np.float16),
                "w2": np.ascontiguousarray(W2[e]).astype(np.float16),
                "b1s": b1se,
            }
        )
    return nc, in_maps, idxs, p0


def _combine(res, idxs, p0, b2):
    b2 = np.asarray(b2, np.float32)
    out = np.zeros((T, H), np.float32)
    for e in range(E):
        ix = idxs[e]
        ye = res.results[e]["yt"][:, : len(ix)].T  # [n_e, H]
        out[ix] += p0[ix, None] * (ye + b2[e][None, :])
    return out.reshape(B, S, H)


def kernel(x, Wr, br, W1, b1, W2, b2):
    from concourse.bass_utils import run_bass_kernel_spmd

    nc, in_maps, idxs, p0 = _prepare(x, Wr, br, W1, b1, W2, b2)
    try:
        res = run_bass_kernel_spmd(nc, in_maps, list(range(E)))
    except Exception:
        import time as _time

        _time.sleep(10)
        res = run_bass_kernel_spmd(nc, in_maps, list(range(E)))
    return _combine(res, idxs, p0, b2)


# revision 6
# speedup vs baseline: 1.0895x; 1.0372x over previous
"""Expert-parallel MoE layer for Trainium2 (8 NeuronCores, one expert per core).

Host side (numpy): router logits, exact top-2 dispatch, p0 weights, and the
scatter-add combine. Device side (Bass/Tile, SPMD over 8 cores): the dense FFN
y = gelu(x @ W1[e] + b1[e]) @ W2[e] over the tokens routed to expert e,
computed with fp16 operands (fp32 PSUM accumulation).

v2 layout: tokens ride the MOVING dim of BOTH GEMMs, so the per-core token
capacity is the exact max expert load (no 128-row padding), and GEMM2 consumes
h^T [F, tokens] directly, producing y^T [H, tokens] written to DRAM exactly
once (no DRAM read-modify-write accumulation like v1's quartered scheme).
Full W1 and W2 stay SBUF-resident in fp16 (128 KB/partition of the 224 KB).

Per token group (TT=512, tail-sized last group):
  GEMM1: psum[f128, t] = sum_k w1[k, f128]^T x^T[k, t]   (8 k-chunks over H)
  gelu+bias -> h[f128-chunk, t] fp16                     (32 f-chunks)
  GEMM2: psum[h'128, t] = sum_k2 w2[k2, h'128]^T h[k2, t] (32 k2-chunks over F)
  copy -> y^T stage -> single DMA store per group
"""

import numpy as np

B, S, H, E, F = 4, 2048, 1024, 8, 4096
T = B * S
P = 128
TT = 512            # token group size (moving free dim of both GEMMs)
KH = H // P         # 8  k-chunks over H  (GEMM1 contraction)
KF = F // P         # 32 k-chunks over F  (GEMM2 contraction)
NH = H // P         # 8  output h'-chunks of GEMM2

_cache = {}


def _spill_waits(nc, mybir, max_waits=1):
    """walrus CoreV2/V3 codegen rejects instructions with >1 semaphore wait
    ("Too many sync wait commands"). Move excess waits onto same-engine no-ops
    inserted right before the instruction (sequencers run in order, so this is
    equivalent)."""
    for fn in nc.m.functions:
        for blk in fn.blocks:
            out = []
            changed = False
            for inst in blk.instructions:
                si = getattr(inst, "sync_info", None)
                if si is not None and len(si.on_wait) > max_waits:
                    spill = si.on_wait[: len(si.on_wait) - max_waits]
                    keep = si.on_wait[len(si.on_wait) - max_waits:]
                    for w in spill:
                        nop = mybir.InstNoOp(
                            name=nc.get_next_instruction_name(),
                            engine=inst.engine,
                            ins=[],
                            outs=[],
                        )
                        nop.sync_info = mybir.SyncInfo(on_wait=[w], on_update=[])
                        out.append(nop)
                    inst.sync_info = mybir.SyncInfo(on_wait=keep, on_update=si.on_update)
                    changed = True
                out.append(inst)
            if changed:
                blk.instructions = out


def _build(cap):
    import concourse.bass as bass
    import concourse.mybir as mybir
    from concourse import tile

    F32 = mybir.dt.float32
    SDT = mybir.dt.float16
    GELU = mybir.ActivationFunctionType.Gelu_apprx_tanh

    nc = bass.Bass()
    xt = nc.declare_dram_parameter("xt", [H, cap], SDT, isOutput=False)
    w1 = nc.declare_dram_parameter("w1", [H, F], SDT, isOutput=False)
    w2 = nc.declare_dram_parameter("w2", [F, H], SDT, isOutput=False)
    b1s = nc.declare_dram_parameter("b1s", [P, KF], F32, isOutput=False)
    yt = nc.declare_dram_parameter("yt", [H, cap], F32, isOutput=True)

    # token groups: full TT groups first, tail last (w1/w2 stream during g0)
    groups = []
    o = 0
    while o < cap:
        tt = min(TT, cap - o)
        groups.append((o, tt))
        o += tt

    xsrc = xt.rearrange("(c p) t -> p c t", p=P)
    w1src = w1.rearrange("(c p) f -> p c f", p=P)
    w2src = w2.rearrange("(c p) h -> p c h", p=P)
    ydst = yt.rearrange("(c p) t -> p c t", p=P)

    with tile.TileContext(nc) as tc:
        with (
            tc.tile_pool(name="w1p", bufs=1) as w1p,
            tc.tile_pool(name="w2p", bufs=1) as w2p,
            tc.tile_pool(name="xp", bufs=1) as xp,
            tc.tile_pool(name="hp", bufs=1) as hp,
            tc.tile_pool(name="yp", bufs=4) as yp,
            tc.tile_pool(name="cst", bufs=1) as cst,
            tc.tile_pool(name="ps1", bufs=4, space="PSUM") as ps1,
            tc.tile_pool(name="ps2", bufs=4, space="PSUM") as ps2,
        ):
            # Startup DMA orchestration. Two constraints drive the layout:
            # (1) Tile hands DMA-completion semaphore lanes out of a shared
            # pool of 8 -- a huge DMA parks a lane for its whole transfer and
            # stalls later DMAs that need the lane back; (2) each dma_start
            # costs ~0.7us of the ISSUING engine's sequencer, and the scalar
            # engine must be free to run gelu as soon as GEMM1 psum group 0
            # lands. So: scalar issues only the 4 x loads (k-halves of group
            # 0 first, so the first matmul unblocks after 512KB), sync issues
            # bias + w1 + w2 in consumption order -- fs-pair chunks for w1's
            # first quarter (fs group 0 unblocks after 512KB), quarters after,
            # then w2 (first needed by GEMM2(g0) ~55us in).
            w1r = w1p.tile([P, KH, F], SDT, tag="w1r")
            x_all = xp.tile([P, KH, cap], SDT, tag="x")
            b1t = cst.tile([P, KF], F32)
            w2r = w2p.tile([P, KF, H], SDT, tag="w2r")

            nc.scalar.dma_start(x_all[:, :4, :TT], xsrc[:, :4, :TT])
            nc.scalar.dma_start(x_all[:, 4:, :TT], xsrc[:, 4:, :TT])
            nc.sync.dma_start(b1t[:], b1s[:])
            FQ = F // 4
            for fp in range(4):  # first quarter in fs-pair chunks
                nc.sync.dma_start(
                    w1r[:, :, fp * 256:(fp + 1) * 256],
                    w1src[:, :, fp * 256:(fp + 1) * 256],
                )
            for q in range(1, 4):
                nc.sync.dma_start(
                    w1r[:, :, q * FQ:(q + 1) * FQ], w1src[:, :, q * FQ:(q + 1) * FQ]
                )
            rest = (cap - TT + 1) // 2
            nc.scalar.dma_start(
                x_all[:, :, TT:TT + rest], xsrc[:, :, TT:TT + rest]
            )
            nc.scalar.dma_start(
                x_all[:, :, TT + rest:], xsrc[:, :, TT + rest:]
            )
            for kc in range(0, KF, 8):
                nc.sync.dma_start(
                    w2r[:, kc:kc + 8, :], w2src[:, kc:kc + 8, :]
                )

            h = hp.tile([P, KF, TT], SDT, tag="h")
            for gi, (t0, tt) in enumerate(groups):
                # GEMM1: h^T[f, t] = gelu(sum_k W1[k, f] * x^T[k, t] + b1[f])
                for fs in range(KF):
                    pt = ps1.tile([P, TT], F32, tag="pt1")
                    for k in range(KH):
                        nc.tensor.matmul(
                            pt[:, :tt],
                            w1r[:, k, fs * P:(fs + 1) * P],
                            x_all[:, k, t0:t0 + tt],
                            start=(k == 0),
                            stop=(k == KH - 1),
                        )
                    nc.scalar.activation(
                        h[:, fs, :tt], pt[:, :tt], GELU, bias=b1t[:, fs:fs + 1]
                    )
                # GEMM2: y^T[h', t] = sum_k2 W2[k2, h'] * h^T[k2, t]
                # per-h'-chunk staging+store so the tail drains during the
                # last copies (and the stage stays at 4x2KB of SBUF)
                for n in range(NH):
                    pt2 = ps2.tile([P, TT], F32, tag="pt2")
                    for k2 in range(KF):
                        nc.tensor.matmul(
                            pt2[:, :tt],
                            w2r[:, k2, n * P:(n + 1) * P],
                            h[:, k2, :tt],
                            start=(k2 == 0),
                            stop=(k2 == KF - 1),
                        )
                    stage = yp.tile([P, TT], F32, tag="stage")
                    nc.vector.tensor_copy(stage[:, :tt], pt2[:, :tt])
                    nc.sync.dma_start(
                        ydst[:, n, t0:t0 + tt], stage[:, :tt]
                    )

    import concourse.mybir as mybir_mod

    _spill_waits(nc, mybir_mod)
    return nc


def _route(x2d, Wr, br):
    """Top-2 routing, bit-matching the reference's decisions.

    Softmax is monotonic, so top-2-of-probs == top-2-of-logits, and the
    normalized top-1 weight p0 = p1/(p1+p2) == sigmoid(l1-l2) exactly (the
    softmax denominator cancels). Ordering ties are broken by lower index,
    same as jax.lax.top_k."""
    logits = x2d @ np.asarray(Wr, np.float32) + np.asarray(br, np.float32)
    order = np.argsort(-logits, axis=-1, kind="stable")
    i1 = order[:, 0].astype(np.int64)
    i2 = order[:, 1].astype(np.int64)
    r = np.arange(logits.shape[0])
    l1 = logits[r, i1].astype(np.float64)
    l2 = logits[r, i2].astype(np.float64)
    p0 = 1.0 / (1.0 + np.exp(l2 - l1))
    return i1, i2, p0.astype(np.float32)


def _prepare(x, Wr, br, W1, b1, W2, b2):
    """Route on host, build per-core input maps and the (cached) Bass program."""
    x2d = np.ascontiguousarray(np.asarray(x, np.float32).reshape(T, H))
    W1 = np.asarray(W1, np.float32)
    b1 = np.asarray(b1, np.float32)
    W2 = np.asarray(W2, np.float32)

    i1, i2, p0 = _route(x2d, Wr, br)
    idxs = [np.flatnonzero((i1 == e) | (i2 == e)) for e in range(E)]
    cap = max(len(ix) for ix in idxs)

    if cap not in _cache:
        _cache[cap] = _build(cap)
    nc = _cache[cap]

    xT = np.ascontiguousarray(x2d.T)  # [H, T]
    in_maps = []
    for e in range(E):
        ix = idxs[e]
        xte = np.zeros((H, cap), np.float16)
        xte[:, : len(ix)] = xT[:, ix]
        b1se = np.ascontiguousarray(b1[e].reshape(KF, P).T)
        in_maps.append(
            {
                "xt": xte,
                "w1": np.ascontiguousarray(W1[e]).astype(np.float16),
                "w2": np.ascontiguousarray(W2[e]).astype(np.float16),
                "b1s": b1se,
            }
        )
    return nc, in_maps, idxs, p0


def _combine(res, idxs, p0, b2):
    b2 = np.asarray(b2, np.float32)
    out = np.zeros((T, H), np.float32)
    for e in range(E):
        ix = idxs[e]
        ye = res.results[e]["yt"][:, : len(ix)].T  # [n_e, H]
        out[ix] += p0[ix, None] * (ye + b2[e][None, :])
    return out.reshape(B, S, H)


def kernel(x, Wr, br, W1, b1, W2, b2):
    from concourse.bass_utils import run_bass_kernel_spmd

    nc, in_maps, idxs, p0 = _prepare(x, Wr, br, W1, b1, W2, b2)
    try:
        res = run_bass_kernel_spmd(nc, in_maps, list(range(E)))
    except Exception:
        import time as _time

        _time.sleep(10)
        res = run_bass_kernel_spmd(nc, in_maps, list(range(E)))
    return _combine(res, idxs, p0, b2)
